# revision 19
# baseline (speedup 1.0000x reference)
"""Multi-head attention (B=4, S=2048, D=1024, H=16) on 8 trn2 NeuronCores.

Sharding: data-parallel over batch (4) x tensor-parallel over heads (2 groups
of 8 heads).  Core c handles batch b=c//2, head group g=c%2: it gets
Wq/Wk/Wv[:, g*512:(g+1)*512] and Wo[g*512:(g+1)*512, :] and produces a partial
output [S, D]; the host sums the two partials of each batch (the row-split of
Wo makes the full output an exact sum of the two group partials).

v2 design (ACT-bound schedule, bf16 operands):
  * host pre-transposes x -> xT [D, S] and casts x/W to bf16: no on-device
    PE-transpose phase, half the input DMA bytes.
  * the key mask is folded into V (V rows and the ones-column are scaled by
    m_k), which is exactly equivalent to the -1e9 score bias and removes the
    per-k-tile bias operand from the exp -> exp batches can span k-tiles.
  * scores are written to PSUM as bf16 (1024/bank), so a 2-ktile score batch
    (2 heads x 2 kt x 512 q = 2048 elems) fits one PSUM bank; exp runs as
    N=2048 ACT instructions, ping-ponged across 2 banks.
  * Q/K/V projections and the y = outT.T @ wo output matmuls are emitted as
    "slack units" interleaved between attention batches so the ~220us of
    softmax exp on the scalar engine overlaps nearly all PE work.
  * per (qc, pt) pass: 8 batches of [2x scores pair-matmuls (row-group
    concurrent), one N=2048 exp, 4 PV matmuls accumulating outT[65, 512]
    (64 dims + masked-ones row-sum column)].  Normalization: recip via
    exp(-ln(x)) on ACT, DRAM-bounce partition-broadcast, in-place bf16
    normalize of outT, then y matmuls run as slack units in later passes.
"""

import os
import sys

import numpy as np

_TRN_REPO = "/opt/trn_rl_repo"
if _TRN_REPO not in sys.path:
    sys.path.insert(0, _TRN_REPO)

from contextlib import ExitStack

import ml_dtypes

import concourse.bass as bass
import concourse.mybir as mybir
import concourse.tile as tile
from concourse import library_config
from concourse.bass_utils import run_bass_kernel_spmd

# If BASS_TRACE is set in the environment, run_bass_kernel_spmd imports
# antenv.axon_hooks, which this container image lacks -- pre-install a stub
# so kernel() degrades to an untraced run instead of crashing.  test.py
# overwrites the stub with a real ctypes-backed hook for profiling.
if "antenv.axon_hooks" not in sys.modules:
    try:
        import antenv.axon_hooks  # noqa: F401
    except Exception:
        import types as _types

        _hookmod = _types.ModuleType("antenv.axon_hooks")
        _hookstore = {}
        _hookmod.set_axon_ntff_profile_hook = lambda h: _hookstore.__setitem__(
            "h", h
        )
        _hookmod.get_axon_ntff_profile_hook = lambda: _hookstore.get("h")
        sys.modules["antenv.axon_hooks"] = _hookmod
        try:
            import antenv

            antenv.axon_hooks = _hookmod
        except Exception:
            pass

S, D, H, DK = 2048, 1024, 16, 64
NCORES = 8
HG = 2                # head-parallel groups
B = 4                 # batches
H8 = H // HG          # heads per core
C = H8 * DK           # 512: per-core projection width
P = 128
KT = D // P           # 8  k-tiles over D
ST = S // P           # 16 tiles over S
CT = C // P           # 4  tiles over C
VW = DK + 1           # 65: v columns + masked-ones column
QC = 512              # q-chunk (one (qc, pt) pass handles 512 queries)
NQC = S // QC

f32 = mybir.dt.float32
bf16 = mybir.dt.bfloat16
i32 = mybir.dt.int32
FT = mybir.ActivationFunctionType
ALU = mybir.AluOpType


def build_nc(split_waits=True):
    nc = bass.Bass()
    xt_d = nc.declare_dram_parameter("xt", [D, S], bf16, isOutput=False)
    wq_d = nc.declare_dram_parameter("wq", [D, C], bf16, isOutput=False)
    wk_d = nc.declare_dram_parameter("wk", [D, C], bf16, isOutput=False)
    wv_d = nc.declare_dram_parameter("wv", [D, C], bf16, isOutput=False)
    wo_d = nc.declare_dram_parameter("wo", [C, D], bf16, isOutput=False)
    mask_d = nc.declare_dram_parameter("maskt", [P, ST], i32, isOutput=False)
    y_d = nc.declare_dram_parameter("y", [S, D], f32, isOutput=True)

    with tile.TileContext(nc) as tc, ExitStack() as ctx:
        perm = ctx.enter_context(tc.tile_pool(name="perm", bufs=1))

        # ---- persistent SBUF tensors
        xT = perm.tile([P, KT, S], bf16)
        wk_sb = perm.tile([P, KT, C], bf16)
        wq_sb = perm.tile([P, KT, C], bf16)
        wv_sb = perm.tile([P, KT, C], bf16)
        wo_sb = perm.tile([P, CT, D], bf16)
        QT = perm.tile([P, CT, S], bf16)
        KTl = perm.tile([P, CT, S], bf16)
        V4 = perm.tile([P, ST, H8, VW], bf16)
        outT = perm.tile([P, CT, S], bf16)
        # 32 (head, q-chunk) row-sum vectors packed at start partitions
        # {0,32,64,96} x 8 column blocks (engine SBUF APs must start at k*32)
        rowsums = perm.tile([P, H8 * NQC // 4, QC], f32)
        nc.vector.memset(rowsums[:, :, :], 1.0)

        # mask as float column per s-tile: keys on partitions
        mask_i = perm.tile([P, ST], i32)
        nc.sync.dma_start(mask_i, mask_d[:, :])
        mcol = perm.tile([P, ST], f32)
        nc.vector.tensor_copy(mcol, mask_i)

        # DMAs split across two queues so weights and xT transfer in
        # parallel, in first-use order: V units need wv + xT chunks, then
        # wk/wq for the K/Q prologue units, wo last (y starts ~100us in).
        xt_r = xt_d.rearrange("(kt p) s -> p kt s", p=P)

        def xt_chunk(eng, sch):
            eng.dma_start(
                xT[:, :, sch * 512 : (sch + 1) * 512],
                xt_r[:, :, sch * 512 : (sch + 1) * 512],
            )

        xt_chunk(nc.sync, 0)
        nc.gpsimd.dma_start(wv_sb, wv_d.rearrange("(kt p) c -> p kt c", p=P))
        xt_chunk(nc.sync, 1)
        nc.gpsimd.dma_start(wk_sb, wk_d.rearrange("(kt p) c -> p kt c", p=P))
        xt_chunk(nc.sync, 2)
        nc.gpsimd.dma_start(wq_sb, wq_d.rearrange("(kt p) c -> p kt c", p=P))
        xt_chunk(nc.gpsimd, 3)
        nc.sync.dma_start(wo_sb, wo_d.rearrange("(pt p) e -> p pt e", p=P))

        # masked-ones column of V4 (denominator = sum_k m_k exp(s_k))
        for st in range(ST):
            nc.vector.tensor_copy(
                V4[:, st, :, DK : DK + 1],
                mcol[:, st : st + 1, None].to_broadcast((P, H8, 1)),
            )

        # ---- PSUM pools: scores 2x2 + PV 2 + proj/y 2 = 8 banks
        scp = ctx.enter_context(
            tc.tile_pool(name="scps", bufs=2, space="PSUM")
        )
        otp = ctx.enter_context(
            tc.tile_pool(name="otps", bufs=2, space="PSUM")
        )
        ppp = ctx.enter_context(
            tc.tile_pool(name="pjps", bufs=2, space="PSUM")
        )
        exp_pool = ctx.enter_context(tc.tile_pool(name="expool", bufs=3))
        bcp = ctx.enter_context(tc.tile_pool(name="bcp", bufs=4))
        ypl = ctx.enter_context(tc.tile_pool(name="ypool", bufs=4))
        rsd = ctx.enter_context(tc.tile_pool(name="rsd", bufs=2, space="DRAM"))

        # ---- slack-unit emitters (each ~8 matmuls + a DVE copy)
        def k_unit(ct, sch):
            ps = ppp.tile([P, C], f32, tag="mm")
            for kt in range(KT):
                nc.tensor.matmul(
                    ps,
                    wk_sb[:, kt, ct * P : (ct + 1) * P],
                    xT[:, kt, sch * 512 : (sch + 1) * 512],
                    start=(kt == 0),
                    stop=(kt == KT - 1),
                )
            nc.vector.tensor_copy(KTl[:, ct, sch * 512 : (sch + 1) * 512], ps)

        def q_unit(ct, qc):
            ps = ppp.tile([P, C], f32, tag="mm")
            for kt in range(KT):
                nc.tensor.matmul(
                    ps,
                    wq_sb[:, kt, ct * P : (ct + 1) * P],
                    xT[:, kt, qc * 512 : (qc + 1) * 512],
                    start=(kt == 0),
                    stop=(kt == KT - 1),
                )
            # fold the 1/sqrt(dk) softmax scale into QT
            nc.vector.tensor_scalar_mul(
                QT[:, ct, qc * 512 : (qc + 1) * 512], ps, 0.125
            )

        def v_unit(st):
            ps = ppp.tile([P, C], f32, tag="mm")
            for kt in range(KT):
                nc.tensor.matmul(
                    ps,
                    xT[:, kt, st * P : (st + 1) * P],
                    wv_sb[:, kt, :],
                    start=(kt == 0),
                    stop=(kt == KT - 1),
                )
            # mask keys while converting PSUM->bf16 V
            nc.vector.tensor_mul(
                V4[:, st, :, 0:DK],
                ps.rearrange("p (h w) -> p h w", w=DK),
                mcol[:, st : st + 1, None].to_broadcast((P, H8, DK)),
            )

        def y_unit(qc, g):
            stg = qc * (QC // P) + g // 2
            ec = g % 2
            ps = ppp.tile([P, 512], f32, tag="mm")
            for pt in range(CT):
                nc.tensor.matmul(
                    ps,
                    outT[:, pt, stg * P : (stg + 1) * P],
                    wo_sb[:, pt, ec * 512 : (ec + 1) * 512],
                    start=(pt == 0),
                    stop=(pt == CT - 1),
                )
            y_sb = ypl.tile([P, 512], f32, tag="y")
            nc.vector.tensor_copy(y_sb, ps)
            nc.sync.dma_start(
                y_d[stg * P : (stg + 1) * P, ec * 512 : (ec + 1) * 512], y_sb
            )

        def norm_pair(qc, pp):
            # normalize outT for head-pairs pt = 2pp, 2pp+1 of q-chunk qc
            # (their 4 row-sum vectors live in rowsums block 2qc+pp): recip
            # via exp(-ln(x)) on ACT, DRAM-bounce partition-broadcast, then
            # in-place bf16 multiply.  Emitted right after pass (qc, 2pp+1).
            blk = 2 * qc + pp
            rsp = rowsums[:, blk : blk + 1, :]
            nc.scalar.activation(rsp, rsp, FT.Ln)
            nc.scalar.activation(rsp, rsp, FT.Exp, scale=-1.0)
            rs_dram = rsd.tile([4, QC], f32, tag="rsd")
            for hh in range(4):
                eng = nc.sync if hh % 2 == 0 else nc.gpsimd
                eng.dma_start(
                    rs_dram[hh : hh + 1, :],
                    rowsums[hh * 32 : hh * 32 + 1, blk, :],
                )
            qs = slice(qc * QC, (qc + 1) * QC)
            for pt in (2 * pp, 2 * pp + 1):
                bc = bcp.tile([P, QC], f32, tag="bc")
                for half in range(2):
                    hh = 2 * (pt - 2 * pp) + half
                    eng = nc.sync if half == 0 else nc.gpsimd
                    eng.dma_start(
                        bc[half * DK : (half + 1) * DK, :],
                        rs_dram[hh : hh + 1, :].to_broadcast((DK, QC)),
                    )
                nc.vector.tensor_mul(outT[:, pt, qs], outT[:, pt, qs], bc)

        # ---- slack-work schedule: pass index p = 4*qc + pt (16 passes,
        # 16 k-tile slots each).  Units listed for pass p are drained one
        # per slot during pass p (deadline: start of some later pass).
        # Pass 0 carries V[5..15] (V[st] must complete before slot st+1
        # consumes it in PV) interleaved with the K/Q units pass 1 needs.
        slack = {p: [] for p in range(16)}
        p0 = [lambda st=st: v_unit(st) for st in (6, 7, 8)]
        for i, st in enumerate((9, 10, 11, 12)):
            p0.append(lambda i=i: k_unit(1, i))
            p0.append(lambda st=st: v_unit(st))
        p0.append(lambda: q_unit(1, 0))
        p0 += [lambda st=st: v_unit(st) for st in (13, 14, 15)]
        slack[0] = p0
        for ct in range(2, CT):
            for sch in range(4):
                slack[ct - 1].append(lambda ct=ct, sch=sch: k_unit(ct, sch))
            slack[ct - 1].append(lambda ct=ct: q_unit(ct, 0))
        for qc in range(1, NQC):
            for ct in range(CT):
                p = 4 * qc + ct - 1
                slack[p].append(lambda ct=ct, qc=qc: q_unit(ct, qc))
        tail_warm = []
        for qc in range(NQC - 1):
            for g in range(8):
                if qc == NQC - 2 and g >= 6:
                    # reserve: emitted after pass 15 to keep PE busy (HAM
                    # warm) while the tail normalization runs
                    tail_warm.append(lambda qc=qc, g=g: y_unit(qc, g))
                else:
                    slack[4 * (qc + 1) + g // 2].append(
                        lambda qc=qc, g=g: y_unit(qc, g)
                    )

        # ---- prologue: V[0..5] interleaved with K pair 0 + Q pair 0 chunk 0
        # (attention pass 0 starts after ~11 units; V[6..15] stream in as
        # pass-0 slack ahead of their PV use)
        v_unit(0)
        v_unit(1)
        for sch in range(4):
            k_unit(0, sch)
            v_unit(2 + sch)
        q_unit(0, 0)

        # ---- attention passes
        for p in range(16):
            qc, pt = p // 4, p % 4
            qs = slice(qc * QC, (qc + 1) * QC)
            units = list(slack[p])
            ot0 = otp.tile([VW, QC], f32, tag="ot")
            ot1 = otp.tile([VW, QC], f32, tag="ot")
            prev = None
            for kt in range(ST):
                sc = scp.tile([P, 2, QC], f32, tag="sc")
                nc.tensor.matmul(
                    sc[:, 0, :],
                    KTl[0:DK, pt, kt * P : (kt + 1) * P],
                    QT[0:DK, pt, qs],
                    start=True,
                    stop=True,
                    tile_position=(0, 0),
                )
                nc.tensor.matmul(
                    sc[:, 1, :],
                    KTl[DK:P, pt, kt * P : (kt + 1) * P],
                    QT[DK:P, pt, qs],
                    start=True,
                    stop=True,
                    tile_position=(64, 0),
                )
                ex = exp_pool.tile([P, 2, QC], bf16, tag="ex")
                nc.scalar.activation(
                    ex.rearrange("p a b -> p (a b)"),
                    sc.rearrange("p a b -> p (a b)"),
                    FT.Exp,
                )
                # norm must be emitted before any dependent y slack unit
                # drains (emission order defines the dependency order)
                if kt == 0:
                    if pt == 2:
                        norm_pair(qc, 0)
                    elif pt == 0 and qc > 0:
                        norm_pair(qc - 1, 1)
                if prev is not None:
                    pk, pex = prev
                    nc.tensor.matmul(
                        ot0,
                        V4[:, pk, 2 * pt, :],
                        pex[:, 0, :],
                        start=(pk == 0),
                        stop=False,
                    )
                    nc.tensor.matmul(
                        ot1,
                        V4[:, pk, 2 * pt + 1, :],
                        pex[:, 1, :],
                        start=(pk == 0),
                        stop=False,
                    )
                if units:
                    units.pop(0)()
                prev = (kt, ex)
            pk, pex = prev
            nc.tensor.matmul(
                ot0, V4[:, pk, 2 * pt, :], pex[:, 0, :], start=False, stop=True
            )
            nc.tensor.matmul(
                ot1,
                V4[:, pk, 2 * pt + 1, :],
                pex[:, 1, :],
                start=False,
                stop=True,
            )
            for u in units:
                u()
            # pass end: row-sum vectors + outT (bf16) copies
            for half, ot in ((0, ot0), (1, ot1)):
                h = 2 * pt + half
                nc.vector.tensor_copy(
                    rowsums[
                        (h % 4) * 32 : (h % 4) * 32 + 1, 2 * qc + h // 4, :
                    ],
                    ot[DK : DK + 1, :],
                )
                nc.vector.tensor_copy(
                    outT[half * DK : (half + 1) * DK, pt, qs], ot[0:DK, :]
                )

        for u in tail_warm:
            u()
        norm_pair(NQC - 1, 1)
        for g in range(8):
            y_unit(NQC - 1, g)

    if split_waits:
        _split_matmul_waits(nc)
    return nc


def _split_matmul_waits(nc):
    """fp32/f32r matmuls (and DMA descriptors) lower to structs that hold
    only ONE sync wait; move extra waits onto a nop on the same engine."""
    import bass_rust

    n = 0
    for f in nc.m.functions:
        for blk in f.blocks:
            out = []
            for inst in blk.instructions:
                si = getattr(inst, "sync_info", None)
                if si is not None and len(si.on_wait) > 1:
                    waits = list(si.on_wait)
                    for w in waits[:-1]:
                        nop = bass_rust.InstNoOp(
                            name=f"I-mmw{n}", ins=[], outs=[], engine=inst.engine
                        )
                        n += 1
                        nop.sync_info = bass_rust.SyncInfo(
                            on_wait=[w], on_update=[]
                        )
                        out.append(nop)
                    inst.sync_info = bass_rust.SyncInfo(
                        on_wait=waits[-1:], on_update=list(si.on_update)
                    )
                out.append(inst)
            blk.instructions = out
    return nc


_NC_CACHE = None


def get_nc():
    global _NC_CACHE
    if _NC_CACHE is None:
        _NC_CACHE = build_nc()
    return _NC_CACHE


def make_in_maps(inputs):
    bf = ml_dtypes.bfloat16
    inp = np.asarray(inputs["inputs"], dtype=np.float32)
    mask = np.asarray(inputs["mask"], dtype=np.int32)
    Wq = np.asarray(inputs["Wq"], dtype=np.float32)
    Wk = np.asarray(inputs["Wk"], dtype=np.float32)
    Wv = np.asarray(inputs["Wv"], dtype=np.float32)
    Wo = np.asarray(inputs["Wo"], dtype=np.float32)

    in_maps = []
    for c in range(NCORES):
        b, g = c // HG, c % HG
        cs = slice(g * C, (g + 1) * C)
        in_maps.append(
            {
                "xt": np.ascontiguousarray(inp[b].T.astype(bf)),
                "wq": np.ascontiguousarray(Wq[:, cs].astype(bf)),
                "wk": np.ascontiguousarray(Wk[:, cs].astype(bf)),
                "wv": np.ascontiguousarray(Wv[:, cs].astype(bf)),
                "wo": np.ascontiguousarray(Wo[cs, :].astype(bf)),
                "maskt": np.ascontiguousarray(mask[b].reshape(ST, P).T),
            }
        )
    return in_maps


def gather(results):
    out = np.empty((B, S, D), np.float32)
    for b in range(B):
        out[b] = results[HG * b]["y"] + results[HG * b + 1]["y"]
    return out


def run(inputs, **kwargs):
    """Run on hardware; returns (output, BassKernelResults)."""
    res = run_bass_kernel_spmd(
        get_nc(), make_in_maps(inputs), list(range(NCORES)), **kwargs
    )
    return gather(res.results), res


def kernel(**inputs) -> np.ndarray:
    out, _ = run(inputs)
    return out


# revision 21
# speedup vs baseline: 1.1872x; 1.1872x over previous
"""Multi-head attention (B=4, S=2048, D=1024, H=16) on 8 trn2 NeuronCores.

Sharding: data-parallel over batch (4) x tensor-parallel over heads (2 groups
of 8 heads).  Core c handles batch b=c//2, head group g=c%2: it gets
Wq/Wk/Wv[:, g*512:(g+1)*512] and Wo[g*512:(g+1)*512, :] and produces a partial
output [S, D]; the host sums the two partials of each batch (the row-split of
Wo makes the full output an exact sum of the two group partials).

v2 design (ACT-bound schedule, bf16 operands):
  * host pre-transposes x -> xT [D, S] and casts x/W to bf16: no on-device
    PE-transpose phase, half the input DMA bytes.
  * the key mask is folded into V (V rows and the ones-column are scaled by
    m_k), which is exactly equivalent to the -1e9 score bias and removes the
    per-k-tile bias operand from the exp -> exp batches can span k-tiles.
  * scores are written to PSUM as bf16 (1024/bank), so a 2-ktile score batch
    (2 heads x 2 kt x 512 q = 2048 elems) fits one PSUM bank; exp runs as
    N=2048 ACT instructions, ping-ponged across 2 banks.
  * Q/K/V projections and the y = outT.T @ wo output matmuls are emitted as
    "slack units" interleaved between attention batches so the ~220us of
    softmax exp on the scalar engine overlaps nearly all PE work.
  * per (qc, pt) pass: 8 batches of [2x scores pair-matmuls (row-group
    concurrent), one N=2048 exp, 4 PV matmuls accumulating outT[65, 512]
    (64 dims + masked-ones row-sum column)].  Normalization: recip via
    exp(-ln(x)) on ACT, DRAM-bounce partition-broadcast, in-place bf16
    normalize of outT, then y matmuls run as slack units in later passes.
"""

import os
import sys

import numpy as np

_TRN_REPO = "/opt/trn_rl_repo"
if _TRN_REPO not in sys.path:
    sys.path.insert(0, _TRN_REPO)

from contextlib import ExitStack

import ml_dtypes

import concourse.bass as bass
import concourse.mybir as mybir
import concourse.tile as tile
from concourse import library_config
from concourse.bass_utils import run_bass_kernel_spmd

# If BASS_TRACE is set in the environment, run_bass_kernel_spmd imports
# antenv.axon_hooks, which this container image lacks -- pre-install a stub
# so kernel() degrades to an untraced run instead of crashing.  test.py
# overwrites the stub with a real ctypes-backed hook for profiling.
if "antenv.axon_hooks" not in sys.modules:
    try:
        import antenv.axon_hooks  # noqa: F401
    except Exception:
        import types as _types

        _hookmod = _types.ModuleType("antenv.axon_hooks")
        _hookstore = {}
        _hookmod.set_axon_ntff_profile_hook = lambda h: _hookstore.__setitem__(
            "h", h
        )
        _hookmod.get_axon_ntff_profile_hook = lambda: _hookstore.get("h")
        sys.modules["antenv.axon_hooks"] = _hookmod
        try:
            import antenv

            antenv.axon_hooks = _hookmod
        except Exception:
            pass

S, D, H, DK = 2048, 1024, 16, 64
NCORES = 8
HG = 2                # head-parallel groups
B = 4                 # batches
H8 = H // HG          # heads per core
C = H8 * DK           # 512: per-core projection width
P = 128
KT = D // P           # 8  k-tiles over D
ST = S // P           # 16 tiles over S
CT = C // P           # 4  tiles over C
VW = DK + 1           # 65: v columns + masked-ones column
QC = 512              # q-chunk (one (qc, pt) pass handles 512 queries)
NQC = S // QC

f32 = mybir.dt.float32
bf16 = mybir.dt.bfloat16
i32 = mybir.dt.int32
FT = mybir.ActivationFunctionType
ALU = mybir.AluOpType


def build_nc(split_waits=True):
    nc = bass.Bass()
    xt_d = nc.declare_dram_parameter("xt", [D, S], bf16, isOutput=False)
    wq_d = nc.declare_dram_parameter("wq", [D, C], bf16, isOutput=False)
    wk_d = nc.declare_dram_parameter("wk", [D, C], bf16, isOutput=False)
    wv_d = nc.declare_dram_parameter("wv", [D, C], bf16, isOutput=False)
    wo_d = nc.declare_dram_parameter("wo", [C, D], bf16, isOutput=False)
    mask_d = nc.declare_dram_parameter("maskt", [P, ST], i32, isOutput=False)
    y_d = nc.declare_dram_parameter("y", [S, D], f32, isOutput=True)

    with tile.TileContext(nc) as tc, ExitStack() as ctx:
        perm = ctx.enter_context(tc.tile_pool(name="perm", bufs=1))

        # ---- persistent SBUF tensors
        xT = perm.tile([P, KT, S], bf16)
        wk_sb = perm.tile([P, KT, C], bf16)
        wq_sb = perm.tile([P, KT, C], bf16)
        wv_sb = perm.tile([P, KT, C], bf16)
        wo_sb = perm.tile([P, CT, D], bf16)
        QT = perm.tile([P, CT, S], bf16)
        KTl = perm.tile([P, CT, S], bf16)
        V4 = perm.tile([P, ST, H8, VW], bf16)
        outT = perm.tile([P, CT, S], bf16)
        # 32 (head, q-chunk) row-sum vectors packed at start partitions
        # {0,32,64,96} x 8 column blocks (engine SBUF APs must start at k*32)
        rowsums = perm.tile([P, H8 * NQC // 4, QC], f32)
        nc.vector.memset(rowsums[:, :, :], 1.0)

        # mask as float column per s-tile: keys on partitions
        mask_i = perm.tile([P, ST], i32)
        nc.sync.dma_start(mask_i, mask_d[:, :])
        mcol = perm.tile([P, ST], f32)
        nc.vector.tensor_copy(mcol, mask_i)

        # DMAs on the sync hardware queue ONLY (gpsimd-queue DMAs are
        # software-DGE and steal SBUF bandwidth from the compute engines),
        # interleaved in first-use order: wv+xT0 gate V[0..3], wk gates K
        # pair 0, etc.  wo last (y starts ~100us in).
        xt_r = xt_d.rearrange("(kt p) s -> p kt s", p=P)

        def xt_chunk(sch):
            nc.sync.dma_start(
                xT[:, :, sch * 512 : (sch + 1) * 512],
                xt_r[:, :, sch * 512 : (sch + 1) * 512],
            )

        nc.sync.dma_start(wv_sb, wv_d.rearrange("(kt p) c -> p kt c", p=P))
        xt_chunk(0)
        nc.sync.dma_start(wk_sb, wk_d.rearrange("(kt p) c -> p kt c", p=P))
        xt_chunk(1)
        nc.sync.dma_start(wq_sb, wq_d.rearrange("(kt p) c -> p kt c", p=P))
        xt_chunk(2)
        xt_chunk(3)
        nc.sync.dma_start(wo_sb, wo_d.rearrange("(pt p) e -> p pt e", p=P))

        # masked-ones column of V4 (denominator = sum_k m_k exp(s_k))
        for st in range(ST):
            nc.vector.tensor_copy(
                V4[:, st, :, DK : DK + 1],
                mcol[:, st : st + 1, None].to_broadcast((P, H8, 1)),
            )

        # ---- PSUM pools: scores 2x2 + PV 2 + proj/y 2 = 8 banks
        scp = ctx.enter_context(
            tc.tile_pool(name="scps", bufs=2, space="PSUM")
        )
        otp = ctx.enter_context(
            tc.tile_pool(name="otps", bufs=2, space="PSUM")
        )
        ppp = ctx.enter_context(
            tc.tile_pool(name="pjps", bufs=2, space="PSUM")
        )
        exp_pool = ctx.enter_context(tc.tile_pool(name="expool", bufs=3))
        bcp = ctx.enter_context(tc.tile_pool(name="bcp", bufs=4))
        ypl = ctx.enter_context(tc.tile_pool(name="ypool", bufs=4))
        rsd = ctx.enter_context(tc.tile_pool(name="rsd", bufs=2, space="DRAM"))

        # ---- slack-unit emitters (each ~8 matmuls + a DVE copy)
        def k_unit(ct, sch):
            ps = ppp.tile([P, C], f32, tag="mm")
            for kt in range(KT):
                nc.tensor.matmul(
                    ps,
                    wk_sb[:, kt, ct * P : (ct + 1) * P],
                    xT[:, kt, sch * 512 : (sch + 1) * 512],
                    start=(kt == 0),
                    stop=(kt == KT - 1),
                )
            nc.vector.tensor_copy(KTl[:, ct, sch * 512 : (sch + 1) * 512], ps)

        def q_unit(ct, qc):
            ps = ppp.tile([P, C], f32, tag="mm")
            for kt in range(KT):
                nc.tensor.matmul(
                    ps,
                    wq_sb[:, kt, ct * P : (ct + 1) * P],
                    xT[:, kt, qc * 512 : (qc + 1) * 512],
                    start=(kt == 0),
                    stop=(kt == KT - 1),
                )
            # fold the 1/sqrt(dk) softmax scale into QT
            nc.vector.tensor_scalar_mul(
                QT[:, ct, qc * 512 : (qc + 1) * 512], ps, 0.125
            )

        def v_unit(st):
            ps = ppp.tile([P, C], f32, tag="mm")
            for kt in range(KT):
                nc.tensor.matmul(
                    ps,
                    xT[:, kt, st * P : (st + 1) * P],
                    wv_sb[:, kt, :],
                    start=(kt == 0),
                    stop=(kt == KT - 1),
                )
            # mask keys while converting PSUM->bf16 V
            nc.vector.tensor_mul(
                V4[:, st, :, 0:DK],
                ps.rearrange("p (h w) -> p h w", w=DK),
                mcol[:, st : st + 1, None].to_broadcast((P, H8, DK)),
            )

        def y_unit(qc, g):
            stg = qc * (QC // P) + g // 2
            ec = g % 2
            ps = ppp.tile([P, 512], f32, tag="mm")
            for pt in range(CT):
                nc.tensor.matmul(
                    ps,
                    outT[:, pt, stg * P : (stg + 1) * P],
                    wo_sb[:, pt, ec * 512 : (ec + 1) * 512],
                    start=(pt == 0),
                    stop=(pt == CT - 1),
                )
            y_sb = ypl.tile([P, 512], f32, tag="y")
            nc.vector.tensor_copy(y_sb, ps)
            nc.sync.dma_start(
                y_d[stg * P : (stg + 1) * P, ec * 512 : (ec + 1) * 512], y_sb
            )

        def norm_pair(qc, pp):
            # normalize outT for head-pairs pt = 2pp, 2pp+1 of q-chunk qc
            # (their 4 row-sum vectors live in rowsums block 2qc+pp): recip
            # via exp(-ln(x)) on ACT, DRAM-bounce partition-broadcast, then
            # in-place bf16 multiply.  Emitted right after pass (qc, 2pp+1).
            blk = 2 * qc + pp
            rsp = rowsums[:, blk : blk + 1, :]
            nc.scalar.activation(rsp, rsp, FT.Ln)
            nc.scalar.activation(rsp, rsp, FT.Exp, scale=-1.0)
            rs_dram = rsd.tile([4, QC], f32, tag="rsd")
            for hh in range(4):
                nc.sync.dma_start(
                    rs_dram[hh : hh + 1, :],
                    rowsums[hh * 32 : hh * 32 + 1, blk, :],
                )
            qs = slice(qc * QC, (qc + 1) * QC)
            for pt in (2 * pp, 2 * pp + 1):
                bc = bcp.tile([P, QC], f32, tag="bc")
                for half in range(2):
                    hh = 2 * (pt - 2 * pp) + half
                    nc.sync.dma_start(
                        bc[half * DK : (half + 1) * DK, :],
                        rs_dram[hh : hh + 1, :].to_broadcast((DK, QC)),
                    )
                nc.vector.tensor_mul(outT[:, pt, qs], outT[:, pt, qs], bc)

        # ---- slack-work schedule: pass index p = 4*qc + pt (16 passes,
        # 16 k-tile slots each).  Units listed for pass p are drained one
        # per slot during pass p (deadline: start of some later pass).
        # Pass 0 carries V[5..15] (V[st] must complete before slot st+1
        # consumes it in PV) interleaved with the K/Q units pass 1 needs.
        slack = {p: [] for p in range(16)}
        p0 = [lambda st=st: v_unit(st) for st in (6, 7, 8)]
        for i, st in enumerate((9, 10, 11, 12)):
            p0.append(lambda i=i: k_unit(1, i))
            p0.append(lambda st=st: v_unit(st))
        p0.append(lambda: q_unit(1, 0))
        p0 += [lambda st=st: v_unit(st) for st in (13, 14, 15)]
        slack[0] = p0
        for ct in range(2, CT):
            for sch in range(4):
                slack[ct - 1].append(lambda ct=ct, sch=sch: k_unit(ct, sch))
            slack[ct - 1].append(lambda ct=ct: q_unit(ct, 0))
        for qc in range(1, NQC):
            for ct in range(CT):
                p = 4 * qc + ct - 1
                slack[p].append(lambda ct=ct, qc=qc: q_unit(ct, qc))
        tail_warm = []
        for qc in range(NQC - 1):
            for g in range(8):
                if qc == NQC - 2 and g >= 6:
                    # reserve: emitted after pass 15 to keep PE busy (HAM
                    # warm) while the tail normalization runs
                    tail_warm.append(lambda qc=qc, g=g: y_unit(qc, g))
                else:
                    slack[4 * (qc + 1) + g // 2].append(
                        lambda qc=qc, g=g: y_unit(qc, g)
                    )

        # ---- prologue: V[0..5] interleaved with K pair 0 + Q pair 0 chunk 0
        # (attention pass 0 starts after ~11 units; V[6..15] stream in as
        # pass-0 slack ahead of their PV use)
        v_unit(0)
        v_unit(1)
        for sch in range(4):
            k_unit(0, sch)
            v_unit(2 + sch)
        q_unit(0, 0)

        # ---- attention passes
        for p in range(16):
            qc, pt = p // 4, p % 4
            qs = slice(qc * QC, (qc + 1) * QC)
            units = list(slack[p])
            ot0 = otp.tile([VW, QC], f32, tag="ot")
            ot1 = otp.tile([VW, QC], f32, tag="ot")
            prev = None
            for kt in range(ST):
                sc = scp.tile([P, 2, QC], f32, tag="sc")
                nc.tensor.matmul(
                    sc[:, 0, :],
                    KTl[0:DK, pt, kt * P : (kt + 1) * P],
                    QT[0:DK, pt, qs],
                    start=True,
                    stop=True,
                    tile_position=(0, 0),
                )
                nc.tensor.matmul(
                    sc[:, 1, :],
                    KTl[DK:P, pt, kt * P : (kt + 1) * P],
                    QT[DK:P, pt, qs],
                    start=True,
                    stop=True,
                    tile_position=(64, 0),
                )
                ex = exp_pool.tile([P, 2, QC], bf16, tag="ex")
                nc.scalar.activation(
                    ex.rearrange("p a b -> p (a b)"),
                    sc.rearrange("p a b -> p (a b)"),
                    FT.Exp,
                )
                # norm must be emitted before any dependent y slack unit
                # drains (emission order defines the dependency order)
                if kt == 0:
                    if pt == 2:
                        norm_pair(qc, 0)
                    elif pt == 0 and qc > 0:
                        norm_pair(qc - 1, 1)
                if prev is not None:
                    pk, pex = prev
                    nc.tensor.matmul(
                        ot0,
                        V4[:, pk, 2 * pt, :],
                        pex[:, 0, :],
                        start=(pk == 0),
                        stop=False,
                    )
                    nc.tensor.matmul(
                        ot1,
                        V4[:, pk, 2 * pt + 1, :],
                        pex[:, 1, :],
                        start=(pk == 0),
                        stop=False,
                    )
                if units:
                    units.pop(0)()
                prev = (kt, ex)
            pk, pex = prev
            nc.tensor.matmul(
                ot0, V4[:, pk, 2 * pt, :], pex[:, 0, :], start=False, stop=True
            )
            nc.tensor.matmul(
                ot1,
                V4[:, pk, 2 * pt + 1, :],
                pex[:, 1, :],
                start=False,
                stop=True,
            )
            for u in units:
                u()
            # pass end: row-sum vectors + outT (bf16) copies
            for half, ot in ((0, ot0), (1, ot1)):
                h = 2 * pt + half
                nc.vector.tensor_copy(
                    rowsums[
                        (h % 4) * 32 : (h % 4) * 32 + 1, 2 * qc + h // 4, :
                    ],
                    ot[DK : DK + 1, :],
                )
                nc.vector.tensor_copy(
                    outT[half * DK : (half + 1) * DK, pt, qs], ot[0:DK, :]
                )

        for u in tail_warm:
            u()
        norm_pair(NQC - 1, 1)
        for g in range(8):
            y_unit(NQC - 1, g)

    if split_waits:
        _split_matmul_waits(nc)
    return nc


def _split_matmul_waits(nc):
    """fp32/f32r matmuls (and DMA descriptors) lower to structs that hold
    only ONE sync wait; move extra waits onto a nop on the same engine."""
    import bass_rust

    n = 0
    for f in nc.m.functions:
        for blk in f.blocks:
            out = []
            for inst in blk.instructions:
                si = getattr(inst, "sync_info", None)
                if si is not None and len(si.on_wait) > 1:
                    waits = list(si.on_wait)
                    for w in waits[:-1]:
                        nop = bass_rust.InstNoOp(
                            name=f"I-mmw{n}", ins=[], outs=[], engine=inst.engine
                        )
                        n += 1
                        nop.sync_info = bass_rust.SyncInfo(
                            on_wait=[w], on_update=[]
                        )
                        out.append(nop)
                    inst.sync_info = bass_rust.SyncInfo(
                        on_wait=waits[-1:], on_update=list(si.on_update)
                    )
                out.append(inst)
            blk.instructions = out
    return nc


_NC_CACHE = None


def get_nc():
    global _NC_CACHE
    if _NC_CACHE is None:
        _NC_CACHE = build_nc()
    return _NC_CACHE


def make_in_maps(inputs):
    bf = ml_dtypes.bfloat16
    inp = np.asarray(inputs["inputs"], dtype=np.float32)
    mask = np.asarray(inputs["mask"], dtype=np.int32)
    Wq = np.asarray(inputs["Wq"], dtype=np.float32)
    Wk = np.asarray(inputs["Wk"], dtype=np.float32)
    Wv = np.asarray(inputs["Wv"], dtype=np.float32)
    Wo = np.asarray(inputs["Wo"], dtype=np.float32)

    in_maps = []
    for c in range(NCORES):
        b, g = c // HG, c % HG
        cs = slice(g * C, (g + 1) * C)
        in_maps.append(
            {
                "xt": np.ascontiguousarray(inp[b].T.astype(bf)),
                "wq": np.ascontiguousarray(Wq[:, cs].astype(bf)),
                "wk": np.ascontiguousarray(Wk[:, cs].astype(bf)),
                "wv": np.ascontiguousarray(Wv[:, cs].astype(bf)),
                "wo": np.ascontiguousarray(Wo[cs, :].astype(bf)),
                "maskt": np.ascontiguousarray(mask[b].reshape(ST, P).T),
            }
        )
    return in_maps


def gather(results):
    out = np.empty((B, S, D), np.float32)
    for b in range(B):
        out[b] = results[HG * b]["y"] + results[HG * b + 1]["y"]
    return out


def run(inputs, **kwargs):
    """Run on hardware; returns (output, BassKernelResults)."""
    res = run_bass_kernel_spmd(
        get_nc(), make_in_maps(inputs), list(range(NCORES)), **kwargs
    )
    return gather(res.results), res


def kernel(**inputs) -> np.ndarray:
    out, _ = run(inputs)
    return out
